# revision 15
# baseline (speedup 1.0000x reference)
"""AttentionPooling (segment softmax + weighted segment-sum) on 8 trn2 cores.

Strategy: shard nodes across cores at segment-aligned cuts (batch is sorted).
Host pre-casts x to bf16, appends the two ones-columns (for the Z row sums),
and pre-transposes a second copy, so the device streams both x [n, d+2]
(pooling values) and xT [d, n] (MLP operand) from HBM with fully contiguous
descriptors and no on-chip transposes. Per 128-node tile: MLP scores (bf16
matmuls, fp32 PSUM) -> exp -> scatter-matmul (A_e^T @ x) accumulating
[64seg, 256+2] in PSUM over a 62-tile window. The A_e one-hot build is
batched per 8-tile megagroup in a [p, seg, tile] layout so the DVE runs in
2x mode, and is software-pipelined one megagroup behind the score pipeline
so the tensor engine never waits on ACT/DVE. Windows dump raw [U|Z] rows;
the host chains the straddling-segment carry between windows and normalizes.
No collectives; host scatters the per-window rows into the final [4096, 256]
output.
"""

import ml_dtypes
import numpy as np

BF16 = ml_dtypes.bfloat16

# ---------------------------------------------------------------- constants
N_FULL = 1_000_000
D = 256
H = 128
G = 4096
NCORES = 8
P = 128
SEG = 64                    # segment rows per window

TILES = 992                 # node tiles per core
NC_PAD = TILES * P          # 126976 padded nodes per core
WINDOWS = 16
TPW = TILES // WINDOWS      # 62 tiles per window
WIN_NODES = TPW * P         # 7936
SUPER = 31                  # node tiles per DMA super-tile
SUPERS_PER_WIN = TPW // SUPER   # 2
OUT_ROWS = WINDOWS * SEG    # 1024 rows per core
EPS = 1e-30


def _set_config(tiles, windows, super_):
    """Reconfigure problem tiling (testing only; defaults are production)."""
    global TILES, NC_PAD, WINDOWS, TPW, WIN_NODES, SUPER, SUPERS_PER_WIN, OUT_ROWS
    TILES, WINDOWS, SUPER = tiles, windows, super_
    NC_PAD = TILES * P
    TPW = TILES // WINDOWS
    assert TPW * WINDOWS == TILES
    WIN_NODES = TPW * P
    SUPERS_PER_WIN = TPW // super_
    assert SUPERS_PER_WIN * super_ == TPW
    OUT_ROWS = WINDOWS * SEG
    _NC_CACHE.clear()


# ---------------------------------------------------------------- host plan
def _plan(batch):
    """batch: sorted int array [N]. Returns per-core planning dicts."""
    batch = np.asarray(batch).astype(np.int64).ravel()
    n = batch.shape[0]
    # all segment-start positions (including 0 and n)
    change = np.flatnonzero(np.diff(batch)) + 1
    bounds = np.concatenate([[0], change, [n]])
    cuts = [0]
    for c in range(1, NCORES):
        tgt = c * n // NCORES
        i = np.searchsorted(bounds, tgt)
        lo = bounds[i - 1] if i > 0 else bounds[0]
        hi = bounds[min(i, len(bounds) - 1)]
        cut = int(lo if (tgt - lo) <= (hi - tgt) else hi)
        cuts.append(cut)
    cuts.append(n)
    for i in range(NCORES):
        assert cuts[i] < cuts[i + 1], f"empty core shard {i}: {cuts}"
        assert cuts[i + 1] - cuts[i] <= NC_PAD, (
            f"core {i} shard {cuts[i + 1] - cuts[i]} > NC_PAD={NC_PAD}"
        )

    plans = []
    for c in range(NCORES):
        lo, hi = cuts[c], cuts[c + 1]
        n_c = hi - lo
        local = batch[lo:hi]
        rel = np.full(NC_PAD, -1.0, np.float32)
        bases = np.full(WINDOWS, -1, np.int64)
        for w in range(WINDOWS):
            a = w * WIN_NODES
            b = min((w + 1) * WIN_NODES, n_c)
            if a >= n_c:
                continue
            base = int(local[a])
            bases[w] = base
            r = local[a:b] - base
            assert r.min() >= 0 and r.max() < SEG, (
                f"core {c} window {w}: {SEG} seg rows exceeded (max rel {r.max()})"
            )
            rel[a:b] = r.astype(np.float32)

        last_seg = int(local[-1])
        diffs = np.full(WINDOWS, -1, np.int64)
        valid = []  # (global_seg_start, nrows) per window
        for w in range(WINDOWS):
            if bases[w] < 0:
                valid.append((0, 0))
                continue
            nxt = bases[w + 1] if (w + 1 < WINDOWS and bases[w + 1] >= 0) else -1
            if nxt >= 0:
                diff = int(nxt - bases[w])
                assert 0 < diff < SEG, f"core {c} window {w}: carry diff {diff}"
                diffs[w] = diff
                hi_seg = nxt
            else:
                hi_seg = last_seg + 1
            nrows = hi_seg - int(bases[w])
            assert 0 < nrows <= SEG
            valid.append((int(bases[w]), int(nrows)))

        # rel_seg rearranged so partition p, col t = rel[t*P + p]
        rel_arr = rel.reshape(TILES, P).T.copy()  # [P, TILES]
        plans.append(
            dict(lo=lo, hi=hi, n_c=n_c, rel_arr=rel_arr, diffs=diffs, valid=valid)
        )
    return plans


def _make_in_maps(x, W1, b1, W2, b2, plans):
    x = np.asarray(x)
    xb_full = x.astype(BF16)
    W1b = np.ascontiguousarray(np.asarray(W1, np.float32)).astype(BF16)
    b1f = np.ascontiguousarray(np.asarray(b1, np.float32)).reshape(H, 1)
    W2b = np.repeat(
        np.asarray(W2, np.float32).reshape(H, 1), 2, axis=1
    ).astype(BF16)
    b2f = np.ascontiguousarray(np.asarray(b2, np.float32)).reshape(1, 1)
    nsup = TILES // SUPER
    in_maps = []
    for pl in plans:
        xp = np.zeros((NC_PAD, D + 2), BF16)
        xp[: pl["n_c"], 0:D] = xb_full[pl["lo"] : pl["hi"]]
        xp[:, D:] = BF16(1.0)
        xtf = xp[:, 0:D].T  # [D, NC_PAD] bf16 view
        # per-super packed xT: partition p holds [k0-chunk || k1-chunk], each
        # contiguous, so one DMA per super moves 15.9 KB/partition
        xt = np.ascontiguousarray(
            xtf.reshape(2, P, nsup, SUPER * P).transpose(2, 1, 0, 3)
        ).reshape(nsup, P, 2 * SUPER * P)
        # natural x (+ones cols) in the exact SBUF super-tile layout, so each
        # DMA reads/writes one contiguous chunk per partition
        xr = np.ascontiguousarray(
            xp.reshape(nsup, SUPER, P, D + 2).transpose(0, 2, 1, 3)
        ).reshape(nsup, P, SUPER * (D + 2))
        in_maps.append(
            {
                "x": xr,
                "xt": xt,
                "relseg": pl["rel_arr"].astype(BF16),
                "w1": W1b,
                "b1": b1f,
                "w2": W2b,
                "b2": b2f,
            }
        )
    return in_maps


def _assemble(outs, plans, dtype):
    """outs: per-core [OUT_ROWS, D+1] raw U|Z windows. Chains the
    straddling-segment carry across windows on the host and normalizes."""
    final = np.zeros((G, D), dtype)
    for pl, o in zip(plans, outs):
        o = np.asarray(o, np.float64)
        carry = np.zeros(D + 1, np.float64)
        for w, (g0, nrows) in enumerate(pl["valid"]):
            if not nrows:
                continue
            uz = o[w * SEG : (w + 1) * SEG].copy()
            uz[0] += carry
            diff = int(pl["diffs"][w])
            carry = uz[diff].copy() if diff >= 0 else np.zeros(D + 1)
            final[g0 : g0 + nrows] = (
                uz[:nrows, 0:D] / (uz[:nrows, D : D + 1] + EPS)
            ).astype(dtype)
    return final


def _super_groups():
    """Tile groups within a super-tile: eights then a remainder group."""
    gs = []
    a = 0
    while a < SUPER:
        gn = min(8, SUPER - a)
        gs.append((a, gn))
        a += gn
    return gs


# ------------------------------------------------------------ numpy emulator
def _emulate(inputs):
    """Pure-numpy emulation of the device program (for logic validation)."""
    x = np.asarray(inputs["x"], np.float32)
    W1 = np.asarray(inputs["W1"], np.float32)
    b1 = np.asarray(inputs["b1"], np.float32)
    W2 = np.asarray(inputs["W2"], np.float32)
    b2 = np.asarray(inputs["b2"], np.float32)
    plans = _plan(inputs["batch"])
    in_maps = _make_in_maps(x, W1, b1, W2, b2, plans)
    outs = []
    cols = np.arange(SEG, dtype=np.float32)[None, :]
    nsup = TILES // SUPER
    for im in in_maps:
        xp = (
            np.asarray(im["x"], np.float32)
            .reshape(nsup, P, SUPER, D + 2)
            .transpose(0, 2, 1, 3)
            .reshape(NC_PAD, D + 2)[:, 0:D]
        )
        rel = im["relseg"].T.reshape(-1).astype(np.float32)  # [NC_PAD]
        h = np.tanh(xp @ W1 + b1.reshape(1, H))
        s = (h @ W2[:, 0:1]).ravel() + float(b2.ravel()[0])
        e = np.exp(s)
        out = np.zeros((OUT_ROWS, D + 1), np.float32)
        for w in range(WINDOWS):
            a, b = w * WIN_NODES, (w + 1) * WIN_NODES
            A = (cols == rel[a:b, None]).astype(np.float32) * e[a:b, None]
            out[w * SEG : (w + 1) * SEG, 0:D] = A.T @ xp[a:b]
            out[w * SEG : (w + 1) * SEG, D] = A.sum(axis=0)
        outs.append(out)
    return _assemble(outs, plans, np.float32)


# ------------------------------------------------------------- bass program
_NC_CACHE = {}


def _build_nc():
    if "nc" in _NC_CACHE:
        return _NC_CACHE["nc"]
    import concourse.bacc as bacc
    import concourse.mybir as mybir
    import concourse.tile as tile

    f32 = mybir.dt.float32
    bf16 = mybir.dt.bfloat16
    AF = mybir.ActivationFunctionType
    ALU = mybir.AluOpType

    nc = bacc.Bacc(None, target_bir_lowering=False)

    nsup = TILES // SUPER
    x_d = nc.dram_tensor(
        "x", [nsup, P, SUPER * (D + 2)], bf16, kind="ExternalInput"
    )
    xt_d = nc.dram_tensor(
        "xt", [nsup, P, 2 * SUPER * P], bf16, kind="ExternalInput"
    )
    rel_d = nc.dram_tensor("relseg", [P, TILES], bf16, kind="ExternalInput")
    w1_d = nc.dram_tensor("w1", [D, H], bf16, kind="ExternalInput")
    b1_d = nc.dram_tensor("b1", [H, 1], f32, kind="ExternalInput")
    w2_d = nc.dram_tensor("w2", [H, 2], bf16, kind="ExternalInput")
    b2_d = nc.dram_tensor("b2", [1, 1], f32, kind="ExternalInput")
    out_d = nc.dram_tensor("out", [OUT_ROWS, D + 1], f32, kind="ExternalOutput")

    with tile.TileContext(nc) as tc:
        with (
            tc.tile_pool(name="singles", bufs=1) as singles,
            tc.tile_pool(name="xsup", bufs=4) as xpool,
            tc.tile_pool(name="xtsup", bufs=4) as xt_pool,
            tc.tile_pool(name="hb", bufs=2) as hb_pool,
            tc.tile_pool(name="e", bufs=3) as e_pool,
            tc.tile_pool(name="ae", bufs=4) as ae_pool,
            tc.tile_pool(name="flush", bufs=2) as flush_pool,
            tc.tile_pool(name="ps_h", bufs=2, space="PSUM") as ps_h,
            tc.tile_pool(name="ps_s", bufs=1, space="PSUM") as ps_s,
            tc.tile_pool(name="ps_uz", bufs=2, space="PSUM") as ps_uz,
        ):
            iota_i = singles.tile([P, SEG], mybir.dt.int32)
            nc.gpsimd.iota(iota_i[:], pattern=[[1, SEG]], base=0, channel_multiplier=0)
            iota_b = singles.tile([P, SEG], bf16)
            nc.vector.tensor_copy(out=iota_b[:], in_=iota_i[:])
            # iotaQT[p, q, t] = q, materialized so the ae build's last dim is
            # unit-stride (DVE 2x mode requires it)
            iotaQT = singles.tile([P, SEG, 8], bf16)
            for j in range(8):
                nc.vector.tensor_copy(out=iotaQT[:, :, j], in_=iota_b[:])

            w1_sb = singles.tile([P, 2, H], bf16)
            w1_r = w1_d[:].rearrange("(c k) m -> c k m", c=2)
            nc.sync.dma_start(out=w1_sb[:, 0, :], in_=w1_r[0])
            nc.sync.dma_start(out=w1_sb[:, 1, :], in_=w1_r[1])
            b1_sb = singles.tile([P, 1], f32)
            nc.sync.dma_start(out=b1_sb[:], in_=b1_d[:])
            w2_sb = singles.tile([P, 2], bf16)
            nc.sync.dma_start(out=w2_sb[:], in_=w2_d[:])
            b2_sb = singles.tile([P, 1], f32)
            nc.sync.dma_start(out=b2_sb[:], in_=b2_d[:].to_broadcast([P, 1]))
            rel_sb = singles.tile([P, TILES], bf16)
            nc.sync.dma_start(out=rel_sb[:], in_=rel_d[:])

            gs = _super_groups()

            for w in range(WINDOWS):
                uz_ps = ps_uz.tile([SEG, D + 2], f32)

                def emit_ae(sup_t, sg_, a_, gn_, e_sb_, w=w, uz_ps=uz_ps):
                    g0 = sg_ * SUPER + a_
                    a01 = ae_pool.tile([P, SEG, 8], bf16)
                    nc.vector.tensor_tensor(
                        out=a01[:, :, 0:gn_],
                        in0=iotaQT[:, :, 0:gn_],
                        in1=rel_sb[:, g0 : g0 + gn_]
                        .rearrange("p (o t) -> p o t", o=1)
                        .to_broadcast([P, SEG, gn_]),
                        op=ALU.is_equal,
                    )
                    aeb = ae_pool.tile([P, SEG, 8], bf16)
                    nc.vector.tensor_tensor(
                        out=aeb[:, :, 0:gn_],
                        in0=a01[:, :, 0:gn_],
                        in1=e_sb_[:, 0:gn_]
                        .rearrange("p (o t) -> p o t", o=1)
                        .to_broadcast([P, SEG, gn_]),
                        op=ALU.mult,
                    )
                    for t in range(gn_):
                        slot = a_ + t
                        ti = g0 + t - w * TPW  # tile index within window
                        nc.tensor.matmul(
                            out=uz_ps[:],
                            lhsT=aeb[:, :, t],
                            rhs=sup_t[:, slot, :],
                            start=(ti == 0),
                            stop=(ti == TPW - 1),
                        )

                pending = None
                for si in range(SUPERS_PER_WIN):
                    sg = w * SUPERS_PER_WIN + si
                    sup = xpool.tile([P, SUPER, D + 2], bf16)
                    nc.sync.dma_start(
                        out=sup[:],
                        in_=x_d[sg].rearrange("p (t c) -> p t c", t=SUPER),
                    )
                    sxt = xt_pool.tile([P, 2, SUPER * P], bf16)
                    nc.scalar.dma_start(
                        out=sxt[:],
                        in_=xt_d[sg].rearrange("p (k n) -> p k n", k=2),
                    )

                    for a, gn in gs:
                        # h = tanh(x @ W1 + b1): [hid, gn*128] in one PSUM
                        # tile spanning two banks; matmuls are k-ordered so
                        # W1 is loaded once per chunk per megagroup.
                        h_ps = ps_h.tile([P, 8 * P], f32)
                        halves = [(0, min(gn, 4))]
                        if gn > 4:
                            halves.append((4, gn - 4))
                        for k in range(2):
                            for ha, hn in halves:
                                nc.tensor.matmul(
                                    out=h_ps[:, ha * P : (ha + hn) * P],
                                    lhsT=w1_sb[:, k, :],
                                    rhs=sxt[:, k, (a + ha) * P : (a + ha + hn) * P],
                                    start=(k == 0),
                                    stop=(k == 1),
                                )
                        hb = hb_pool.tile([P, 8 * P], bf16)
                        nc.scalar.activation(
                            out=hb[:, 0 : gn * P],
                            in_=h_ps[:, 0 : gn * P],
                            func=AF.Tanh,
                            bias=b1_sb[:],
                            scale=1.0,
                        )
                        if pending is not None:
                            emit_ae(*pending)
                            pending = None
                        s_ps = ps_s.tile([P, 8, 2], f32)
                        for t in range(gn):
                            nc.tensor.matmul(
                                out=s_ps[:, t, :],
                                lhsT=hb[:, t * P : (t + 1) * P],
                                rhs=w2_sb[:],
                                start=True,
                                stop=True,
                            )
                        e_sb = e_pool.tile([P, 8], bf16)
                        nc.scalar.activation(
                            out=e_sb[:, 0:gn],
                            in_=s_ps[:, 0:gn, 0],
                            func=AF.Exp,
                            bias=b2_sb[:],
                            scale=1.0,
                        )
                        pending = (sup, sg, a, gn, e_sb)
                emit_ae(*pending)

                # ---- flush window w: dump raw U|Z rows; host does the rest
                uz_sb = flush_pool.tile([SEG, D + 1], f32)
                nc.vector.tensor_copy(out=uz_sb[:], in_=uz_ps[:, 0 : D + 1])
                nc.sync.dma_start(
                    out=out_d[w * SEG : (w + 1) * SEG, :], in_=uz_sb[:]
                )

    nc.finalize()
    _NC_CACHE["nc"] = nc
    return nc


def _run(inputs, trace=False):
    from concourse.bass_utils import run_bass_kernel_spmd

    x = inputs["x"]
    plans = _plan(inputs["batch"])
    in_maps = _make_in_maps(
        x, inputs["W1"], inputs["b1"], inputs["W2"], inputs["b2"], plans
    )
    nc = _build_nc()
    res = run_bass_kernel_spmd(
        nc, in_maps, core_ids=list(range(NCORES)), trace=trace
    )
    outs = [r["out"] for r in res.results]
    final = _assemble(outs, plans, np.float32)
    return final, res


def kernel(**inputs):
    return _run(inputs, trace=False)[0]


# revision 16
# speedup vs baseline: 1.0234x; 1.0234x over previous
"""AttentionPooling (segment softmax + weighted segment-sum) on 8 trn2 cores.

Strategy: shard nodes across cores at segment-aligned cuts (batch is sorted).
Host pre-casts x to bf16, appends the two ones-columns (for the Z row sums),
and pre-transposes a second copy, so the device streams both x [n, d+2]
(pooling values) and xT [d, n] (MLP operand) from HBM with fully contiguous
descriptors and no on-chip transposes. Per 128-node tile: MLP scores (bf16
matmuls, fp32 PSUM) -> exp -> scatter-matmul (A_e^T @ x) accumulating
[64seg, 256+2] in PSUM over a 62-tile window. The A_e one-hot build is
batched per 8-tile megagroup in a [p, seg, tile] layout so the DVE runs in
2x mode, and is software-pipelined one megagroup behind the score pipeline
so the tensor engine never waits on ACT/DVE. Windows dump raw [U|Z] rows;
the host chains the straddling-segment carry between windows and normalizes.
No collectives; host scatters the per-window rows into the final [4096, 256]
output.
"""

import ml_dtypes
import numpy as np

BF16 = ml_dtypes.bfloat16

# ---------------------------------------------------------------- constants
N_FULL = 1_000_000
D = 256
H = 128
G = 4096
NCORES = 8
P = 128
SEG = 64                    # segment rows per window

TILES = 992                 # node tiles per core
NC_PAD = TILES * P          # 126976 padded nodes per core
WINDOWS = 16
TPW = TILES // WINDOWS      # 62 tiles per window
WIN_NODES = TPW * P         # 7936
SUPER = 31                  # node tiles per DMA super-tile
SUPERS_PER_WIN = TPW // SUPER   # 2
OUT_ROWS = WINDOWS * SEG    # 1024 rows per core
EPS = 1e-30


def _set_config(tiles, windows, super_):
    """Reconfigure problem tiling (testing only; defaults are production)."""
    global TILES, NC_PAD, WINDOWS, TPW, WIN_NODES, SUPER, SUPERS_PER_WIN, OUT_ROWS
    TILES, WINDOWS, SUPER = tiles, windows, super_
    NC_PAD = TILES * P
    TPW = TILES // WINDOWS
    assert TPW * WINDOWS == TILES
    WIN_NODES = TPW * P
    SUPERS_PER_WIN = TPW // super_
    assert SUPERS_PER_WIN * super_ == TPW
    OUT_ROWS = WINDOWS * SEG
    _NC_CACHE.clear()


# ---------------------------------------------------------------- host plan
def _plan(batch):
    """batch: sorted int array [N]. Returns per-core planning dicts."""
    batch = np.asarray(batch).astype(np.int64).ravel()
    n = batch.shape[0]
    # all segment-start positions (including 0 and n)
    change = np.flatnonzero(np.diff(batch)) + 1
    bounds = np.concatenate([[0], change, [n]])
    cuts = [0]
    for c in range(1, NCORES):
        tgt = c * n // NCORES
        i = np.searchsorted(bounds, tgt)
        lo = bounds[i - 1] if i > 0 else bounds[0]
        hi = bounds[min(i, len(bounds) - 1)]
        cut = int(lo if (tgt - lo) <= (hi - tgt) else hi)
        cuts.append(cut)
    cuts.append(n)
    for i in range(NCORES):
        assert cuts[i] < cuts[i + 1], f"empty core shard {i}: {cuts}"
        assert cuts[i + 1] - cuts[i] <= NC_PAD, (
            f"core {i} shard {cuts[i + 1] - cuts[i]} > NC_PAD={NC_PAD}"
        )

    plans = []
    for c in range(NCORES):
        lo, hi = cuts[c], cuts[c + 1]
        n_c = hi - lo
        local = batch[lo:hi]
        rel = np.full(NC_PAD, -1.0, np.float32)
        bases = np.full(WINDOWS, -1, np.int64)
        for w in range(WINDOWS):
            a = w * WIN_NODES
            b = min((w + 1) * WIN_NODES, n_c)
            if a >= n_c:
                continue
            base = int(local[a])
            bases[w] = base
            r = local[a:b] - base
            assert r.min() >= 0 and r.max() < SEG, (
                f"core {c} window {w}: {SEG} seg rows exceeded (max rel {r.max()})"
            )
            rel[a:b] = r.astype(np.float32)

        last_seg = int(local[-1])
        diffs = np.full(WINDOWS, -1, np.int64)
        valid = []  # (global_seg_start, nrows) per window
        for w in range(WINDOWS):
            if bases[w] < 0:
                valid.append((0, 0))
                continue
            nxt = bases[w + 1] if (w + 1 < WINDOWS and bases[w + 1] >= 0) else -1
            if nxt >= 0:
                diff = int(nxt - bases[w])
                assert 0 < diff < SEG, f"core {c} window {w}: carry diff {diff}"
                diffs[w] = diff
                hi_seg = nxt
            else:
                hi_seg = last_seg + 1
            nrows = hi_seg - int(bases[w])
            assert 0 < nrows <= SEG
            valid.append((int(bases[w]), int(nrows)))

        # rel_seg rearranged so partition p, col t = rel[t*P + p]
        rel_arr = rel.reshape(TILES, P).T.copy()  # [P, TILES]
        plans.append(
            dict(lo=lo, hi=hi, n_c=n_c, rel_arr=rel_arr, diffs=diffs, valid=valid)
        )
    return plans


def _make_in_maps(x, W1, b1, W2, b2, plans):
    x = np.asarray(x)
    xb_full = x.astype(BF16)
    W1b = np.ascontiguousarray(np.asarray(W1, np.float32)).astype(BF16)
    b1f = np.ascontiguousarray(np.asarray(b1, np.float32)).reshape(H, 1)
    W2b = np.repeat(
        np.asarray(W2, np.float32).reshape(H, 1), 2, axis=1
    ).astype(BF16)
    b2f = np.ascontiguousarray(np.asarray(b2, np.float32)).reshape(1, 1)
    nsup = TILES // SUPER
    in_maps = []
    for pl in plans:
        xp = np.zeros((NC_PAD, D + 2), BF16)
        xp[: pl["n_c"], 0:D] = xb_full[pl["lo"] : pl["hi"]]
        xp[:, D:] = BF16(1.0)
        xtf = xp[:, 0:D].T  # [D, NC_PAD] bf16 view
        # per-super packed xT: partition p holds [k0-chunk || k1-chunk], each
        # contiguous, so one DMA per super moves 15.9 KB/partition
        xt = np.ascontiguousarray(
            xtf.reshape(2, P, nsup, SUPER * P).transpose(2, 1, 0, 3)
        ).reshape(nsup, P, 2 * SUPER * P)
        # natural x (+ones cols) in the exact SBUF super-tile layout, so each
        # DMA reads/writes one contiguous chunk per partition
        xr = np.ascontiguousarray(
            xp.reshape(nsup, SUPER, P, D + 2).transpose(0, 2, 1, 3)
        ).reshape(nsup, P, SUPER * (D + 2))
        in_maps.append(
            {
                "x": xr,
                "xt": xt,
                "relseg": pl["rel_arr"].astype(BF16),
                "w1": W1b,
                "b1": b1f,
                "w2": W2b,
                "b2": b2f,
            }
        )
    return in_maps


def _assemble(outs, plans, dtype):
    """outs: per-core [OUT_ROWS, D+1] raw U|Z windows. Chains the
    straddling-segment carry across windows on the host and normalizes."""
    final = np.zeros((G, D), dtype)
    for pl, o in zip(plans, outs):
        o = np.asarray(o, np.float64)
        carry = np.zeros(D + 1, np.float64)
        for w, (g0, nrows) in enumerate(pl["valid"]):
            if not nrows:
                continue
            uz = o[w * SEG : (w + 1) * SEG].copy()
            uz[0] += carry
            diff = int(pl["diffs"][w])
            carry = uz[diff].copy() if diff >= 0 else np.zeros(D + 1)
            final[g0 : g0 + nrows] = (
                uz[:nrows, 0:D] / (uz[:nrows, D : D + 1] + EPS)
            ).astype(dtype)
    return final


def _super_groups():
    """Tile groups within a super-tile: eights then a remainder group."""
    gs = []
    a = 0
    while a < SUPER:
        gn = min(8, SUPER - a)
        gs.append((a, gn))
        a += gn
    return gs


# ------------------------------------------------------------ numpy emulator
def _emulate(inputs):
    """Pure-numpy emulation of the device program (for logic validation)."""
    x = np.asarray(inputs["x"], np.float32)
    W1 = np.asarray(inputs["W1"], np.float32)
    b1 = np.asarray(inputs["b1"], np.float32)
    W2 = np.asarray(inputs["W2"], np.float32)
    b2 = np.asarray(inputs["b2"], np.float32)
    plans = _plan(inputs["batch"])
    in_maps = _make_in_maps(x, W1, b1, W2, b2, plans)
    outs = []
    cols = np.arange(SEG, dtype=np.float32)[None, :]
    nsup = TILES // SUPER
    for im in in_maps:
        xp = (
            np.asarray(im["x"], np.float32)
            .reshape(nsup, P, SUPER, D + 2)
            .transpose(0, 2, 1, 3)
            .reshape(NC_PAD, D + 2)[:, 0:D]
        )
        rel = im["relseg"].T.reshape(-1).astype(np.float32)  # [NC_PAD]
        h = np.tanh(xp @ W1 + b1.reshape(1, H))
        s = (h @ W2[:, 0:1]).ravel() + float(b2.ravel()[0])
        e = np.exp(s)
        out = np.zeros((OUT_ROWS, D + 1), np.float32)
        for w in range(WINDOWS):
            a, b = w * WIN_NODES, (w + 1) * WIN_NODES
            A = (cols == rel[a:b, None]).astype(np.float32) * e[a:b, None]
            out[w * SEG : (w + 1) * SEG, 0:D] = A.T @ xp[a:b]
            out[w * SEG : (w + 1) * SEG, D] = A.sum(axis=0)
        outs.append(out)
    return _assemble(outs, plans, np.float32)


# ------------------------------------------------------------- bass program
_NC_CACHE = {}


def _build_nc():
    if "nc" in _NC_CACHE:
        return _NC_CACHE["nc"]
    import concourse.bacc as bacc
    import concourse.mybir as mybir
    import concourse.tile as tile

    f32 = mybir.dt.float32
    bf16 = mybir.dt.bfloat16
    AF = mybir.ActivationFunctionType
    ALU = mybir.AluOpType

    nc = bacc.Bacc(None, target_bir_lowering=False)

    nsup = TILES // SUPER
    x_d = nc.dram_tensor(
        "x", [nsup, P, SUPER * (D + 2)], bf16, kind="ExternalInput"
    )
    xt_d = nc.dram_tensor(
        "xt", [nsup, P, 2 * SUPER * P], bf16, kind="ExternalInput"
    )
    rel_d = nc.dram_tensor("relseg", [P, TILES], bf16, kind="ExternalInput")
    w1_d = nc.dram_tensor("w1", [D, H], bf16, kind="ExternalInput")
    b1_d = nc.dram_tensor("b1", [H, 1], f32, kind="ExternalInput")
    w2_d = nc.dram_tensor("w2", [H, 2], bf16, kind="ExternalInput")
    b2_d = nc.dram_tensor("b2", [1, 1], f32, kind="ExternalInput")
    out_d = nc.dram_tensor("out", [OUT_ROWS, D + 1], f32, kind="ExternalOutput")

    with tile.TileContext(nc) as tc:
        with (
            tc.tile_pool(name="singles", bufs=1) as singles,
            tc.tile_pool(name="xsup", bufs=5) as xpool,
            tc.tile_pool(name="xtsup", bufs=5) as xt_pool,
            tc.tile_pool(name="hb", bufs=2) as hb_pool,
            tc.tile_pool(name="e", bufs=3) as e_pool,
            tc.tile_pool(name="ae", bufs=4) as ae_pool,
            tc.tile_pool(name="flush", bufs=2) as flush_pool,
            tc.tile_pool(name="ps_h", bufs=2, space="PSUM") as ps_h,
            tc.tile_pool(name="ps_s", bufs=1, space="PSUM") as ps_s,
            tc.tile_pool(name="ps_uz", bufs=2, space="PSUM") as ps_uz,
        ):
            iota_i = singles.tile([P, SEG], mybir.dt.int32)
            nc.gpsimd.iota(iota_i[:], pattern=[[1, SEG]], base=0, channel_multiplier=0)
            iota_b = singles.tile([P, SEG], bf16)
            nc.vector.tensor_copy(out=iota_b[:], in_=iota_i[:])
            # iotaQT[p, q, t] = q, materialized so the ae build's last dim is
            # unit-stride (DVE 2x mode requires it)
            iotaQT = singles.tile([P, SEG, 8], bf16)
            for j in range(8):
                nc.vector.tensor_copy(out=iotaQT[:, :, j], in_=iota_b[:])

            w1_sb = singles.tile([P, 2, H], bf16)
            w1_r = w1_d[:].rearrange("(c k) m -> c k m", c=2)
            nc.sync.dma_start(out=w1_sb[:, 0, :], in_=w1_r[0])
            nc.sync.dma_start(out=w1_sb[:, 1, :], in_=w1_r[1])
            b1_sb = singles.tile([P, 1], f32)
            nc.sync.dma_start(out=b1_sb[:], in_=b1_d[:])
            w2_sb = singles.tile([P, 2], bf16)
            nc.sync.dma_start(out=w2_sb[:], in_=w2_d[:])
            b2_sb = singles.tile([P, 1], f32)
            nc.sync.dma_start(out=b2_sb[:], in_=b2_d[:].to_broadcast([P, 1]))
            rel_sb = singles.tile([P, TILES], bf16)
            nc.sync.dma_start(out=rel_sb[:], in_=rel_d[:])

            gs = _super_groups()

            for w in range(WINDOWS):
                uz_ps = ps_uz.tile([SEG, D + 2], f32)

                def emit_ae(sup_t, sg_, a_, gn_, e_sb_, w=w, uz_ps=uz_ps):
                    g0 = sg_ * SUPER + a_
                    a01 = ae_pool.tile([P, SEG, 8], bf16)
                    nc.vector.tensor_tensor(
                        out=a01[:, :, 0:gn_],
                        in0=iotaQT[:, :, 0:gn_],
                        in1=rel_sb[:, g0 : g0 + gn_]
                        .rearrange("p (o t) -> p o t", o=1)
                        .to_broadcast([P, SEG, gn_]),
                        op=ALU.is_equal,
                    )
                    aeb = ae_pool.tile([P, SEG, 8], bf16)
                    nc.vector.tensor_tensor(
                        out=aeb[:, :, 0:gn_],
                        in0=a01[:, :, 0:gn_],
                        in1=e_sb_[:, 0:gn_]
                        .rearrange("p (o t) -> p o t", o=1)
                        .to_broadcast([P, SEG, gn_]),
                        op=ALU.mult,
                    )
                    for t in range(gn_):
                        slot = a_ + t
                        ti = g0 + t - w * TPW  # tile index within window
                        nc.tensor.matmul(
                            out=uz_ps[:],
                            lhsT=aeb[:, :, t],
                            rhs=sup_t[:, slot, :],
                            start=(ti == 0),
                            stop=(ti == TPW - 1),
                        )

                pending = None
                for si in range(SUPERS_PER_WIN):
                    sg = w * SUPERS_PER_WIN + si
                    sup = xpool.tile([P, SUPER, D + 2], bf16)
                    nc.sync.dma_start(
                        out=sup[:],
                        in_=x_d[sg].rearrange("p (t c) -> p t c", t=SUPER),
                    )
                    sxt = xt_pool.tile([P, 2, SUPER * P], bf16)
                    nc.scalar.dma_start(
                        out=sxt[:],
                        in_=xt_d[sg].rearrange("p (k n) -> p k n", k=2),
                    )

                    for a, gn in gs:
                        # h = tanh(x @ W1 + b1): [hid, gn*128] in one PSUM
                        # tile spanning two banks; matmuls are k-ordered so
                        # W1 is loaded once per chunk per megagroup.
                        h_ps = ps_h.tile([P, 8 * P], f32)
                        halves = [(0, min(gn, 4))]
                        if gn > 4:
                            halves.append((4, gn - 4))
                        for k in range(2):
                            for ha, hn in halves:
                                nc.tensor.matmul(
                                    out=h_ps[:, ha * P : (ha + hn) * P],
                                    lhsT=w1_sb[:, k, :],
                                    rhs=sxt[:, k, (a + ha) * P : (a + ha + hn) * P],
                                    start=(k == 0),
                                    stop=(k == 1),
                                )
                        hb = hb_pool.tile([P, 8 * P], bf16)
                        nc.scalar.activation(
                            out=hb[:, 0 : gn * P],
                            in_=h_ps[:, 0 : gn * P],
                            func=AF.Tanh,
                            bias=b1_sb[:],
                            scale=1.0,
                        )
                        if pending is not None:
                            emit_ae(*pending)
                            pending = None
                        s_ps = ps_s.tile([P, 8, 2], f32)
                        for t in range(gn):
                            nc.tensor.matmul(
                                out=s_ps[:, t, :],
                                lhsT=hb[:, t * P : (t + 1) * P],
                                rhs=w2_sb[:],
                                start=True,
                                stop=True,
                            )
                        e_sb = e_pool.tile([P, 8], bf16)
                        nc.scalar.activation(
                            out=e_sb[:, 0:gn],
                            in_=s_ps[:, 0:gn, 0],
                            func=AF.Exp,
                            bias=b2_sb[:],
                            scale=1.0,
                        )
                        pending = (sup, sg, a, gn, e_sb)
                emit_ae(*pending)

                # ---- flush window w: dump raw U|Z rows; host does the rest
                uz_sb = flush_pool.tile([SEG, D + 1], f32)
                nc.vector.tensor_copy(out=uz_sb[:], in_=uz_ps[:, 0 : D + 1])
                nc.gpsimd.dma_start(
                    out=out_d[w * SEG : (w + 1) * SEG, :], in_=uz_sb[:]
                )

    nc.finalize()
    _NC_CACHE["nc"] = nc
    return nc


def _run(inputs, trace=False):
    from concourse.bass_utils import run_bass_kernel_spmd

    x = inputs["x"]
    plans = _plan(inputs["batch"])
    in_maps = _make_in_maps(
        x, inputs["W1"], inputs["b1"], inputs["W2"], inputs["b2"], plans
    )
    nc = _build_nc()
    res = run_bass_kernel_spmd(
        nc, in_maps, core_ids=list(range(NCORES)), trace=trace
    )
    outs = [r["out"] for r in res.results]
    final = _assemble(outs, plans, np.float32)
    return final, res


def kernel(**inputs):
    return _run(inputs, trace=False)[0]
